# revision 6
# baseline (speedup 1.0000x reference)
"""HMM forward-backward posterior marginals on 8 trn2 NeuronCores.

Math: the emission log-probs are state-independent scalars per (b, t)
(`log_emission_probs[obvs]` broadcast over states), and log_gamma is
normalized over states at the end, so every emission term cancels and the
output is identical for every batch row:

    gamma[b, t, i] = f_t[i] + g_t[i] - LSE_i(f_t + g_t)

with batch-independent scans
    f_t = LSE_j(f_{t-1}[j] + T[j, i]),   f_0 = log_initial_probs
    g_t = LSE_k(g_{t+1}[k] + T[i, k]),   g_{T-1} = 0.

Any per-t additive constant in f_t/g_t cancels in the normalization, so the
scans can run in the exponential domain as plain linear recurrences
    Xf_{t+1} = A^T Xf_t,   Xg_t = A Xg_{t+1},   A = exp(T - kappa)
with kappa = asymptotic growth rate (Perron value), which keeps magnitudes
O(1). Positive-sum matmuls: numerically benign in f32.

Device strategy (identical SPMD program on 8 cores; core c's output is batch
rows 8c..8c+7, which are all equal to h):
  - Host (f64, O(N^2) work): kappa, A, A^T, and chunk-boundary vectors for
    C=512 chunks of L=8 steps via (A^8) matvecs.
  - Device: 7+7 chained 128x128x512 matmuls fill the chunk interiors,
    elementwise Xf*Xg, 32 PE transposes to (t, i) layout, row-normalize via
    reduce/reciprocal, Ln activation, and DMA h out 8x (one per batch row).
"""

import numpy as np

NUM_STATES = 128
SEQ_LEN = 4096
BATCH = 64
N_CORES = 8
B_PER_CORE = BATCH // N_CORES
L = 8               # interior steps per chunk
C = SEQ_LEN // L    # 512 chunks


def _host_prep(log_T, log_pi):
    """f64 host prep: growth rate, exp-domain matrices, boundary vectors."""
    Tm = log_T.astype(np.float64)
    pi = log_pi.astype(np.float64)
    N = Tm.shape[0]

    # Growth rate kappa: iterate the log-domain recurrence to convergence.
    v = np.zeros(N)
    kappa = 0.0
    for _ in range(200):
        m = (v[:, None] + Tm).max(axis=0)
        nv = m + np.log(np.exp(v[:, None] + Tm - m[None, :]).sum(axis=0))
        kappa = nv.max()
        v = nv - kappa

    A = np.exp(Tm - kappa)          # (j, i) forward: x' = A^T x
    A8 = A.copy()
    for _ in range(3):              # A8 = (normalized) A^8
        A8 = A8 @ A8
        A8 /= A8.max()

    # Forward boundaries: Bf[:, c] = exp-domain f at t = c*L, col-max 1.
    Bf = np.empty((N, C))
    x = np.exp(pi - pi.max())
    Bf[:, 0] = x
    for c in range(1, C):
        x = A8.T @ x
        x /= x.max()
        Bf[:, c] = x
    # Backward boundaries: Bg[:, c] = exp-domain g at t = c*L + L - 1.
    Bg = np.empty((N, C))
    y = np.ones(N)
    Bg[:, C - 1] = y
    for c in range(C - 2, -1, -1):
        y = A8 @ y
        y /= y.max()
        Bg[:, c] = y

    f32 = np.float32
    return (A.astype(f32), A.T.astype(f32).copy(),
            Bf.astype(f32), Bg.astype(f32))


N = NUM_STATES
CONST_W = 3 * N + 2 * C  # [a_f | a_g | ident | xfb | xgb] along free dim


def _build_bass():
    import concourse.bacc as bacc
    import concourse.mybir as mybir
    from concourse import tile

    dt = mybir.dt.float32
    nc = bacc.Bacc()

    # All constants in ONE input tensor → one DMA → one semaphore, so PE
    # instructions (whose f32 self-loading LdWeights struct carries at most
    # one sync wait) never need two DMA waits.
    cin = nc.dram_tensor("cin", [N, CONST_W], dt, kind="ExternalInput")
    out = nc.dram_tensor("out", [B_PER_CORE, SEQ_LEN, N], dt,
                         kind="ExternalOutput")

    with tile.TileContext(nc) as tc:
        with (
            tc.tile_pool(name="consts", bufs=1) as consts,
            tc.tile_pool(name="scan", bufs=1) as scan,
            tc.tile_pool(name="ps_scan", bufs=2, space="PSUM") as ps_scan,
            tc.tile_pool(name="ps_tr", bufs=4, space="PSUM") as ps_tr,
            tc.tile_pool(name="stats", bufs=8) as stats,
            tc.tile_pool(name="hout", bufs=6) as hout,
        ):
            cb = consts.tile([N, CONST_W], dt, tag="cb")
            nc.sync.dma_start(cb[:], cin[:])
            t_af = cb[:, 0:N]
            t_ag = cb[:, N:2 * N]
            t_id = cb[:, 2 * N:3 * N]
            xf0 = cb[:, 3 * N:3 * N + C]
            xg0 = cb[:, 3 * N + C:3 * N + 2 * C]

            # xf/xg hold interior steps s=1..L-1; s=0 comes from cb slices.
            xf = scan.tile([N, (L - 1) * C], dt, tag="xf")
            xg = scan.tile([N, (L - 1) * C], dt, tag="xg")
            u = scan.tile([N, SEQ_LEN], dt, tag="u")    # t-major

            def xf_s(s):
                return xf0 if s == 0 else xf[:, (s - 1) * C:s * C]

            def xg_s(s):
                return xg0 if s == 0 else xg[:, (s - 1) * C:s * C]

            # Interior scans: step s from step s-1, one matmul each.
            for s in range(1, L):
                pf = ps_scan.tile([N, C], dt, tag="pf")
                nc.tensor.matmul(pf[:], t_af, xf_s(s - 1))
                nc.vector.tensor_copy(xf[:, (s - 1) * C:s * C], pf[:])
                pg = ps_scan.tile([N, C], dt, tag="pg")
                nc.tensor.matmul(pg[:], t_ag, xg_s(s - 1))
                nc.vector.tensor_copy(xg[:, (s - 1) * C:s * C], pg[:])

            # u[:, c*L + s] = xf_s[:, c] * xg_{L-1-s}[:, c]  (t-major layout)
            u3 = u.rearrange("p (c s) -> p c s", s=L)
            for s in range(L):
                nc.vector.tensor_mul(u3[:, :, s], xf_s(s), xg_s(L - 1 - s))

            # Per 128-t block: transpose to (t, i), normalize rows, Ln, DMA.
            # PSUM tile is read by DVE only (reduce + copy) so the next
            # transpose's WAR dep is a single semaphore.
            n_blocks = SEQ_LEN // N
            for j in range(n_blocks):
                pt = ps_tr.tile([N, N], dt, tag="pt")
                nc.tensor.transpose(pt[:], u[:, j * N:(j + 1) * N], t_id)
                ssum = stats.tile([N, 1], dt, tag="ssum")
                nc.vector.reduce_sum(ssum[:], pt[:], axis=mybir.AxisListType.X)
                ub = hout.tile([N, N], dt, tag="ub")
                nc.vector.tensor_copy(ub[:], pt[:])
                rsum = stats.tile([N, 1], dt, tag="rsum")
                nc.vector.reciprocal(rsum[:], ssum[:])
                hb = hout.tile([N, N], dt, tag="hb")
                nc.scalar.activation(
                    hb[:], ub[:], mybir.ActivationFunctionType.Ln,
                    scale=rsum[:])
                for b in range(B_PER_CORE):
                    nc.sync.dma_start(
                        out[b, j * N:(j + 1) * N, :], hb[:])
    nc.compile()
    return nc


def kernel(obvs, log_initial_probs, log_transition_matrix, log_emission_probs):
    from concourse.bass_utils import run_bass_kernel_spmd

    a_f, a_g, bf, bg = _host_prep(
        np.asarray(log_transition_matrix), np.asarray(log_initial_probs))
    cin = _pack_consts(a_f, a_g, bf, bg)

    nc = _build_bass()
    res = run_bass_kernel_spmd(
        nc, [{"cin": cin} for _ in range(N_CORES)], list(range(N_CORES)))
    return np.concatenate([r["out"] for r in res.results], axis=0)


def _pack_consts(a_f, a_g, bf, bg):
    ident = np.eye(NUM_STATES, dtype=np.float32)
    return np.concatenate([a_f, a_g, ident, bf, bg], axis=1)


# revision 7
# speedup vs baseline: 1.9986x; 1.9986x over previous
"""HMM forward-backward posterior marginals on 8 trn2 NeuronCores.

Math: the emission log-probs are state-independent scalars per (b, t)
(`log_emission_probs[obvs]` broadcast over states), and log_gamma is
normalized over states at the end, so every emission term cancels and the
output is identical for every batch row:

    gamma[b, t, i] = f_t[i] + g_t[i] - LSE_i(f_t + g_t)

with batch-independent scans
    f_t = LSE_j(f_{t-1}[j] + T[j, i]),   f_0 = log_initial_probs
    g_t = LSE_k(g_{t+1}[k] + T[i, k]),   g_{T-1} = 0.

Any per-t additive constant in f_t/g_t cancels in the normalization, so the
scans can run in the exponential domain as plain linear recurrences
    Xf_{t+1} = A^T Xf_t,   Xg_t = A Xg_{t+1},   A = exp(T - kappa)
with kappa = asymptotic growth rate (Perron value), which keeps magnitudes
O(1). Positive-sum matmuls: numerically benign in f32.

Device strategy (identical SPMD program on 8 cores; core c's output is batch
rows 8c..8c+7, which are all equal to h):
  - Host (f64, O(N^2) work): kappa, A, A^T, and chunk-boundary vectors for
    C=512 chunks of L=8 steps via (A^8) matvecs.
  - Device: 7+7 chained 128x128x512 matmuls fill the chunk interiors,
    elementwise Xf*Xg, 32 PE transposes to (t, i) layout, row-normalize via
    reduce/reciprocal, Ln activation, and DMA h out 8x (one per batch row).
"""

import numpy as np

NUM_STATES = 128
SEQ_LEN = 4096
BATCH = 64
N_CORES = 8
B_PER_CORE = BATCH // N_CORES
L = 8               # interior steps per chunk
C = SEQ_LEN // L    # 512 chunks


def _host_prep(log_T, log_pi):
    """f64 host prep: growth rate, exp-domain matrices, boundary vectors."""
    Tm = log_T.astype(np.float64)
    pi = log_pi.astype(np.float64)
    N = Tm.shape[0]

    # Growth rate kappa: iterate the log-domain recurrence to convergence.
    v = np.zeros(N)
    kappa = 0.0
    for _ in range(200):
        m = (v[:, None] + Tm).max(axis=0)
        nv = m + np.log(np.exp(v[:, None] + Tm - m[None, :]).sum(axis=0))
        kappa = nv.max()
        v = nv - kappa

    A = np.exp(Tm - kappa)          # (j, i) forward: x' = A^T x
    A8 = A.copy()
    for _ in range(3):              # A8 = (normalized) A^8
        A8 = A8 @ A8
        A8 /= A8.max()

    # Forward boundaries: Bf[:, c] = exp-domain f at t = c*L, col-max 1.
    Bf = np.empty((N, C))
    x = np.exp(pi - pi.max())
    Bf[:, 0] = x
    for c in range(1, C):
        x = A8.T @ x
        x /= x.max()
        Bf[:, c] = x
    # Backward boundaries: Bg[:, c] = exp-domain g at t = c*L + L - 1.
    Bg = np.empty((N, C))
    y = np.ones(N)
    Bg[:, C - 1] = y
    for c in range(C - 2, -1, -1):
        y = A8 @ y
        y /= y.max()
        Bg[:, c] = y

    f32 = np.float32
    return (A.astype(f32), A.T.astype(f32).copy(),
            Bf.astype(f32), Bg.astype(f32))


N = NUM_STATES
CONST_W = 3 * N + 2 * C  # [a_f | a_g | ident | xfb | xgb] along free dim


def _build_bass():
    import concourse.bacc as bacc
    import concourse.mybir as mybir
    from concourse import tile

    dt = mybir.dt.float32
    nc = bacc.Bacc()

    # All constants in ONE input tensor → one DMA → one semaphore, so PE
    # instructions (whose f32 self-loading LdWeights struct carries at most
    # one sync wait) never need two DMA waits.
    cin = nc.dram_tensor("cin", [N, CONST_W], dt, kind="ExternalInput")
    out = nc.dram_tensor("out", [B_PER_CORE, SEQ_LEN, N], dt,
                         kind="ExternalOutput")

    with tile.TileContext(nc) as tc:
        with (
            tc.tile_pool(name="consts", bufs=1) as consts,
            tc.tile_pool(name="scan", bufs=1) as scan,
            tc.tile_pool(name="ps_scan", bufs=2, space="PSUM") as ps_scan,
            tc.tile_pool(name="ps_tr", bufs=4, space="PSUM") as ps_tr,
            tc.tile_pool(name="stats", bufs=8) as stats,
            tc.tile_pool(name="hout", bufs=6) as hout,
        ):
            cb = consts.tile([N, CONST_W], dt, tag="cb")
            nc.sync.dma_start(cb[:], cin[:])
            t_af = cb[:, 0:N]
            t_ag = cb[:, N:2 * N]
            t_id = cb[:, 2 * N:3 * N]
            xf0 = cb[:, 3 * N:3 * N + C]
            xg0 = cb[:, 3 * N + C:3 * N + 2 * C]

            # xf/xg hold interior steps s=1..L-1; s=0 comes from cb slices.
            xf = scan.tile([N, (L - 1) * C], dt, tag="xf")
            xg = scan.tile([N, (L - 1) * C], dt, tag="xg")
            u = scan.tile([N, SEQ_LEN], dt, tag="u")    # t-major

            def xf_s(s):
                return xf0 if s == 0 else xf[:, (s - 1) * C:s * C]

            def xg_s(s):
                return xg0 if s == 0 else xg[:, (s - 1) * C:s * C]

            # Interior scans: step s from step s-1, one matmul each.
            for s in range(1, L):
                pf = ps_scan.tile([N, C], dt, tag="pf")
                nc.tensor.matmul(pf[:], t_af, xf_s(s - 1))
                nc.vector.tensor_copy(xf[:, (s - 1) * C:s * C], pf[:])
                pg = ps_scan.tile([N, C], dt, tag="pg")
                nc.tensor.matmul(pg[:], t_ag, xg_s(s - 1))
                nc.vector.tensor_copy(xg[:, (s - 1) * C:s * C], pg[:])

            # u[:, c*L + s] = xf_s[:, c] * xg_{L-1-s}[:, c]  (t-major layout)
            u3 = u.rearrange("p (c s) -> p c s", s=L)
            for s in range(L):
                nc.vector.tensor_mul(u3[:, :, s], xf_s(s), xg_s(L - 1 - s))

            # Per 128-t block: transpose to (t, i), normalize rows, Ln into
            # the accumulator Hall; DMA out in big grouped transfers (few
            # dma_start instructions — HWDGE descriptor-gen is ~0.8us each
            # and 256 small DMAs serialize on it).
            n_blocks = SEQ_LEN // N
            hall = scan.tile([N, SEQ_LEN], dt, tag="hall")  # h in (t%128, j*128+i)
            blocks_per_group = 8
            # dst: out[b] viewed as (j p) i -> p j i so one DMA moves a
            # whole group of 8 blocks (512 KB) per batch row.
            out_pji = [
                out[b].rearrange("(j p) i -> p j i", p=N)
                for b in range(B_PER_CORE)
            ]
            for j in range(n_blocks):
                pt = ps_tr.tile([N, N], dt, tag="pt")
                nc.tensor.transpose(pt[:], u[:, j * N:(j + 1) * N], t_id)
                ssum = stats.tile([N, 1], dt, tag="ssum")
                nc.vector.reduce_sum(ssum[:], pt[:], axis=mybir.AxisListType.X)
                rsum = stats.tile([N, 1], dt, tag="rsum")
                nc.vector.reciprocal(rsum[:], ssum[:])
                nc.scalar.activation(
                    hall[:, j * N:(j + 1) * N], pt[:],
                    mybir.ActivationFunctionType.Ln, scale=rsum[:])
                if (j + 1) % blocks_per_group == 0:
                    g0 = (j + 1 - blocks_per_group)
                    for b in range(B_PER_CORE):
                        nc.sync.dma_start(
                            out_pji[b][:, g0:j + 1, :],
                            hall[:, g0 * N:(j + 1) * N])
    nc.compile()
    return nc


def kernel(obvs, log_initial_probs, log_transition_matrix, log_emission_probs):
    from concourse.bass_utils import run_bass_kernel_spmd

    a_f, a_g, bf, bg = _host_prep(
        np.asarray(log_transition_matrix), np.asarray(log_initial_probs))
    cin = _pack_consts(a_f, a_g, bf, bg)

    nc = _build_bass()
    res = run_bass_kernel_spmd(
        nc, [{"cin": cin} for _ in range(N_CORES)], list(range(N_CORES)))
    return np.concatenate([r["out"] for r in res.results], axis=0)


def _pack_consts(a_f, a_g, bf, bg):
    ident = np.eye(NUM_STATES, dtype=np.float32)
    return np.concatenate([a_f, a_g, ident, bf, bg], axis=1)


# revision 8
# speedup vs baseline: 2.0203x; 1.0108x over previous
"""HMM forward-backward posterior marginals on 8 trn2 NeuronCores.

Math: the emission log-probs are state-independent scalars per (b, t)
(`log_emission_probs[obvs]` broadcast over states), and log_gamma is
normalized over states at the end, so every emission term cancels and the
output is identical for every batch row:

    gamma[b, t, i] = f_t[i] + g_t[i] - LSE_i(f_t + g_t)

with batch-independent scans
    f_t = LSE_j(f_{t-1}[j] + T[j, i]),   f_0 = log_initial_probs
    g_t = LSE_k(g_{t+1}[k] + T[i, k]),   g_{T-1} = 0.

Any per-t additive constant in f_t/g_t cancels in the normalization, so the
scans can run in the exponential domain as plain linear recurrences
    Xf_{t+1} = A^T Xf_t,   Xg_t = A Xg_{t+1},   A = exp(T - kappa)
with kappa = asymptotic growth rate (Perron value), which keeps magnitudes
O(1). Positive-sum matmuls: numerically benign in f32.

Device strategy (identical SPMD program on 8 cores; core c's output is batch
rows 8c..8c+7, which are all equal to h):
  - Host (f64, O(N^2) work): kappa, A, A^T, and chunk-boundary vectors for
    C=512 chunks of L=8 steps via (A^8) matvecs.
  - Device: 7+7 chained 128x128x512 matmuls fill the chunk interiors,
    elementwise Xf*Xg, 32 PE transposes to (t, i) layout, row-normalize via
    reduce/reciprocal, Ln activation, and DMA h out 8x (one per batch row).
"""

import numpy as np

NUM_STATES = 128
SEQ_LEN = 4096
BATCH = 64
N_CORES = 8
B_PER_CORE = BATCH // N_CORES
L = 8               # interior steps per chunk
C = SEQ_LEN // L    # 512 chunks


def _host_prep(log_T, log_pi):
    """f64 host prep: growth rate, exp-domain matrices, boundary vectors."""
    Tm = log_T.astype(np.float64)
    pi = log_pi.astype(np.float64)
    N = Tm.shape[0]

    # Growth rate kappa: iterate the log-domain recurrence to convergence.
    v = np.zeros(N)
    kappa = 0.0
    for _ in range(200):
        m = (v[:, None] + Tm).max(axis=0)
        nv = m + np.log(np.exp(v[:, None] + Tm - m[None, :]).sum(axis=0))
        kappa = nv.max()
        v = nv - kappa

    A = np.exp(Tm - kappa)          # (j, i) forward: x' = A^T x
    A8 = A.copy()
    for _ in range(3):              # A8 = (normalized) A^8
        A8 = A8 @ A8
        A8 /= A8.max()

    # Forward boundaries: Bf[:, c] = exp-domain f at t = c*L, col-max 1.
    Bf = np.empty((N, C))
    x = np.exp(pi - pi.max())
    Bf[:, 0] = x
    for c in range(1, C):
        x = A8.T @ x
        x /= x.max()
        Bf[:, c] = x
    # Backward boundaries: Bg[:, c] = exp-domain g at t = c*L + L - 1.
    Bg = np.empty((N, C))
    y = np.ones(N)
    Bg[:, C - 1] = y
    for c in range(C - 2, -1, -1):
        y = A8 @ y
        y /= y.max()
        Bg[:, c] = y

    f32 = np.float32
    return (A.astype(f32), A.T.astype(f32).copy(),
            Bf.astype(f32), Bg.astype(f32))


N = NUM_STATES
CONST_W = 3 * N + 2 * C  # [a_f | a_g | ident | xfb | xgb] along free dim


def _build_bass():
    import concourse.bacc as bacc
    import concourse.mybir as mybir
    from concourse import tile

    dt = mybir.dt.float32
    nc = bacc.Bacc()

    # All constants in ONE input tensor → one DMA → one semaphore, so PE
    # instructions (whose f32 self-loading LdWeights struct carries at most
    # one sync wait) never need two DMA waits.
    cin = nc.dram_tensor("cin", [N, CONST_W], dt, kind="ExternalInput")
    out = nc.dram_tensor("out", [B_PER_CORE, SEQ_LEN, N], dt,
                         kind="ExternalOutput")

    with tile.TileContext(nc) as tc:
        with (
            tc.tile_pool(name="consts", bufs=1) as consts,
            tc.tile_pool(name="scan", bufs=1) as scan,
            tc.tile_pool(name="ps_scan", bufs=2, space="PSUM") as ps_scan,
            tc.tile_pool(name="ps_tr", bufs=4, space="PSUM") as ps_tr,
            tc.tile_pool(name="stats", bufs=8) as stats,
            tc.tile_pool(name="hout", bufs=6) as hout,
        ):
            cb = consts.tile([N, CONST_W], dt, tag="cb")
            nc.sync.dma_start(cb[:], cin[:])
            t_af = cb[:, 0:N]
            t_ag = cb[:, N:2 * N]
            t_id = cb[:, 2 * N:3 * N]
            xf0 = cb[:, 3 * N:3 * N + C]
            xg0 = cb[:, 3 * N + C:3 * N + 2 * C]

            # xf/xg hold interior steps s=1..L-1; s=0 comes from cb slices.
            xf = scan.tile([N, (L - 1) * C], dt, tag="xf")
            xg = scan.tile([N, (L - 1) * C], dt, tag="xg")
            u = scan.tile([N, SEQ_LEN], dt, tag="u")    # t-major

            def xf_s(s):
                return xf0 if s == 0 else xf[:, (s - 1) * C:s * C]

            def xg_s(s):
                return xg0 if s == 0 else xg[:, (s - 1) * C:s * C]

            # Interior scans: step s from step s-1, one matmul each.
            for s in range(1, L):
                pf = ps_scan.tile([N, C], dt, tag="pf")
                nc.tensor.matmul(pf[:], t_af, xf_s(s - 1))
                nc.vector.tensor_copy(xf[:, (s - 1) * C:s * C], pf[:])
                pg = ps_scan.tile([N, C], dt, tag="pg")
                nc.tensor.matmul(pg[:], t_ag, xg_s(s - 1))
                nc.vector.tensor_copy(xg[:, (s - 1) * C:s * C], pg[:])

            # u[:, c*L + s] = xf_s[:, c] * xg_{L-1-s}[:, c]  (t-major layout)
            u3 = u.rearrange("p (c s) -> p c s", s=L)
            for s in range(L):
                nc.vector.tensor_mul(u3[:, :, s], xf_s(s), xg_s(L - 1 - s))

            # Per 128-t block: transpose to (t, i), normalize rows, Ln into
            # the accumulator Hall; DMA out in big grouped transfers (few
            # dma_start instructions — HWDGE descriptor-gen is ~0.8us each
            # and 256 small DMAs serialize on it).
            n_blocks = SEQ_LEN // N
            hall = scan.tile([N, SEQ_LEN], dt, tag="hall")  # h in (t%128, j*128+i)
            blocks_per_group = 8
            # dst: out[b] viewed as (j p) i -> p j i so one DMA moves a
            # whole group of 8 blocks (512 KB) per batch row.
            out_pji = [
                out[b].rearrange("(j p) i -> p j i", p=N)
                for b in range(B_PER_CORE)
            ]
            for j in range(n_blocks):
                pt = ps_tr.tile([N, N], dt, tag="pt")
                nc.tensor.transpose(pt[:], u[:, j * N:(j + 1) * N], t_id)
                ssum = stats.tile([N, 1], dt, tag="ssum")
                nc.vector.reduce_sum(ssum[:], pt[:], axis=mybir.AxisListType.X)
                rsum = stats.tile([N, 1], dt, tag="rsum")
                nc.vector.reciprocal(rsum[:], ssum[:])
                nc.scalar.activation(
                    hall[:, j * N:(j + 1) * N], pt[:],
                    mybir.ActivationFunctionType.Ln, scale=rsum[:])
                if (j + 1) % blocks_per_group == 0:
                    g0 = (j + 1 - blocks_per_group)
                    for b in range(B_PER_CORE):
                        eng = nc.sync if b % 2 == 0 else nc.scalar
                        eng.dma_start(
                            out_pji[b][:, g0:j + 1, :],
                            hall[:, g0 * N:(j + 1) * N])
    nc.compile()
    return nc


def kernel(obvs, log_initial_probs, log_transition_matrix, log_emission_probs):
    from concourse.bass_utils import run_bass_kernel_spmd

    a_f, a_g, bf, bg = _host_prep(
        np.asarray(log_transition_matrix), np.asarray(log_initial_probs))
    cin = _pack_consts(a_f, a_g, bf, bg)

    nc = _build_bass()
    res = run_bass_kernel_spmd(
        nc, [{"cin": cin} for _ in range(N_CORES)], list(range(N_CORES)))
    return np.concatenate([r["out"] for r in res.results], axis=0)


def _pack_consts(a_f, a_g, bf, bg):
    ident = np.eye(NUM_STATES, dtype=np.float32)
    return np.concatenate([a_f, a_g, ident, bf, bg], axis=1)
